# revision 36
# baseline (speedup 1.0000x reference)
"""AnomalyAttention on 8 Trainium2 NeuronCores.

Batch-parallel: core i computes batch element i (B=8). No collectives.
Host-side prep: transpose q/k/v per batch element (plus bf16 copies for
the TensorEngine) and precompute the (i-j)^2 distance table; everything
else (projections, sigma head, scores, softmax, Gaussian prior,
attention, output projection) runs on device.

Matmuls run in bf16 with fp32 PSUM accumulation.  The sigma head
(which the prior is very sensitive to) runs fully in fp32.  series and
prior are produced as bf16 in DRAM and upcast to fp32 on the host.

Outputs (matching the reference tuple): out [8,1024,512],
series [8,8,1024,1024], prior [8,8,1024,1024], all float32.
"""

import math
import sys

import numpy as np

B, W, D, H = 8, 1024, 512, 8
HS = D // H          # 64
P = 128
IC = D // P          # 4 contraction chunks over D
QC = W // P          # 8 row chunks over W
LN3 = math.log(3.0)
LN_SQRT_2PI = 0.9189385332046727  # ln(sqrt(2*pi))


def _ensure_path():
    for p in ("/opt/trn_rl_repo", "/root/.axon_site/_ro/trn_rl_repo"):
        if p not in sys.path:
            sys.path.append(p)


def _patch_tile_drain():
    """Walrus in this container rejects the multi-wait Drain that
    TileContext emits at exit ("Too many sync wait commands").  Split the
    final waits across single-wait NOPs on the sync engine instead."""
    import concourse.tile as tile
    from concourse.vector_clock import ScopedClock, VectorClock

    if getattr(tile.TileContext, "_drain_patched", False):
        return

    def _drain_and_barrier(self, tick_clock, wait_clock):
        vc = tick_clock.global_clock
        for i in range(len(vc)):
            if vc[i] <= 0:
                continue
            sub = VectorClock([vc[j] if j == i else 0 for j in range(len(vc))])
            nop = self.nc.sync.nop(nofuse=True, hint="drain_split")
            wait_clock.add_sem_waits(nop.ins, ScopedClock({None: sub}))
        self.nc.sync.drain()
        self.nc.all_engine_barrier()
        assert self.sems is not None
        popped = self.nc._tile_sem_poison_stack.pop()
        assert popped is self._sem_poison
        self.nc.clear_and_free_semaphores(list(self.sems.allocated().values()))
        self.nc.all_engine_barrier()

    tile.TileContext._drain_and_barrier = _drain_and_barrier
    tile.TileContext._drain_patched = True


def _split_multi_waits(nc):
    """This container's walrus allows at most one sync-wait per
    instruction.  Hoist extra waits onto NOPs inserted just before, on the
    same engine."""
    import concourse.mybir as mybir

    n_split = 0
    for f in nc.m.functions:
        for bb in f.blocks:
            insts = bb.instructions
            i = 0
            while i < len(insts):
                inst = insts[i]
                si = inst.sync_info
                if si is not None and si.on_wait and len(si.on_wait) > 1:
                    waits = list(si.on_wait)
                    nops = []
                    for j, wt in enumerate(waits[:-1]):
                        nop = mybir.InstNoOp(name=f"{inst.name}-wsplit{j}")
                        nop.engine = inst.engine
                        nop.sync_info = mybir.SyncInfo(on_wait=[wt],
                                                       on_update=[])
                        nops.append(nop)
                    inst.sync_info = mybir.SyncInfo(
                        on_wait=[waits[-1]], on_update=list(si.on_update or []))
                    for j, nop in enumerate(nops):
                        insts.insert(i + j, nop)
                    i += len(nops)
                    n_split += 1
                i += 1
    return n_split


def _patch_ldw_opt():
    """Re-enable walrus's LDWEIGHTS dedup (disabled by default in this
    container's compile driver)."""
    import concourse.bass_utils as bu

    if getattr(bu, "_ldw_patched", False):
        return
    orig = bu.run_command

    def run_command(cmd, *a, **kw):
        return orig(cmd, *a, **kw)

    bu.run_command = run_command
    bu._ldw_patched = True


def _dedup_ldweights(nc):
    """Remove back-to-back duplicate LDWEIGHTS on the PE stream (walrus's
    ldw-opt pass is disabled in this container and crashes when enabled).
    Safe only when the duplicate carries no sync conditions."""
    import concourse.mybir as mybir

    n_rm = 0
    for f in nc.m.functions:
        for bb in f.blocks:
            insts = bb.instructions
            last_ldw_key = None
            keep = []
            for inst in insts:
                if inst.engine == mybir.EngineType.PE:
                    op = type(inst).__name__
                    if op == "InstLdweights":
                        try:
                            pa = inst.ins[0]
                            key = (pa.memref, pa.offset, str(pa.ap),
                                   str(pa.dtype), str(inst.perf_mode),
                                   str(inst.is_transpose))
                        except Exception:
                            key = None
                        si = inst.sync_info
                        clean = not (si and (si.on_wait or si.on_update))
                        if clean and key is not None and key == last_ldw_key:
                            n_rm += 1
                            continue  # drop duplicate
                        last_ldw_key = key
                    elif op != "InstMatmult":
                        last_ldw_key = None
                keep.append(inst)
            if len(keep) != len(insts):
                insts[:] = keep
    return n_rm


def build_nc():
    """Build the single-core Bass graph (SPMD: identical on all 8 cores)."""
    _ensure_path()
    _patch_tile_drain()
    _patch_ldw_opt()

    import concourse.bass as bass
    import concourse.mybir as mybir
    import concourse.tile as tile
    from concourse.masks import make_identity

    F32 = mybir.dt.float32
    BF = mybir.dt.float16       # TensorEngine operand dtype (full rate, 10-bit mantissa)
    BFO = mybir.dt.bfloat16     # series/prior DRAM output dtype (wide exponent)
    ACTF = mybir.ActivationFunctionType

    nc = bass.Bass()

    # fp32 sigma path input
    qT_d = nc.declare_dram_parameter("qT", [D, W], F32, isOutput=False)
    # bf16 TensorEngine inputs
    qTb_d = nc.declare_dram_parameter("qTb", [D, W], BF, isOutput=False)
    kTb_d = nc.declare_dram_parameter("kTb", [D, W], BF, isOutput=False)
    vTb_d = nc.declare_dram_parameter("vTb", [D, W], BF, isOutput=False)
    Wq_d = nc.declare_dram_parameter("Wq", [D, D], BF, isOutput=False)
    Wk_d = nc.declare_dram_parameter("Wk", [D, D], BF, isOutput=False)
    Wv_d = nc.declare_dram_parameter("Wv", [D, D], BF, isOutput=False)
    Wo_d = nc.declare_dram_parameter("Wo", [D, D], BF, isOutput=False)
    Wsig_d = nc.declare_dram_parameter("Wsig", [D, H], F32, isOutput=False)
    bq_d = nc.declare_dram_parameter("bq", [D], F32, isOutput=False)
    bk_d = nc.declare_dram_parameter("bk", [D], F32, isOutput=False)
    bo_d = nc.declare_dram_parameter("bo", [1, D], BF, isOutput=False)
    bsig_d = nc.declare_dram_parameter("bsig", [1, H], F32, isOutput=False)
    dist2_d = nc.declare_dram_parameter("dist2", [W, W], F32, isOutput=False)
    ones32_d = nc.declare_dram_parameter("ones32", [1, P], F32, isOutput=False)
    onesb_d = nc.declare_dram_parameter("onesb", [1, P], BF, isOutput=False)

    out_d = nc.declare_dram_parameter("out", [W, D], F32, isOutput=True)
    series_d = nc.declare_dram_parameter("series", [H, W, W], BFO, isOutput=True)
    prior_d = nc.declare_dram_parameter("prior", [H, W, W], BFO, isOutput=True)

    mm = nc.tensor.matmul

    # prior is exactly zero (f32 underflow) for |i-j| > ~29 (sigma <= 2.00003);
    # compute only a +-64 band per 128-row chunk, the rest of each row stays
    # at the pre-zeroed output buffer value.
    def band(ic):
        jlo = max(0, ic * P - HS)
        jhi = min(W, (ic + 1) * P + HS)
        return jlo, jhi - jlo

    with tile.TileContext(nc) as tc:
        with (
            tc.tile_pool(name="const", bufs=1) as cpool,
            tc.tile_pool(name="acts", bufs=1) as apool,
        ):
            # ---- persistent constants ----
            Wq_s = cpool.tile([P, IC, D], BF)
            Wk_s = cpool.tile([P, IC, D], BF)
            Wv_s = cpool.tile([P, IC, D], BF)
            Wo_s = cpool.tile([P, IC, D], BF)
            Wsig_s = cpool.tile([P, IC, H], F32)
            for t, d in ((Wq_s, Wq_d), (Wk_s, Wk_d), (Wv_s, Wv_d), (Wo_s, Wo_d),
                         (Wsig_s, Wsig_d)):
                nc.sync.dma_start(t[:], d.rearrange("(c p) f -> p c f", p=P))
            bq_s = cpool.tile([P, IC], F32)
            bk_s = cpool.tile([P, IC], F32)
            nc.sync.dma_start(bq_s[:], bq_d.rearrange("(c p) -> p c", p=P))
            nc.sync.dma_start(bk_s[:], bk_d.rearrange("(c p) -> p c", p=P))
            bo_row = cpool.tile([1, D], BF)
            bsig_row = cpool.tile([1, H], F32)
            nc.sync.dma_start(bo_row[:], bo_d[:])
            nc.sync.dma_start(bsig_row[:], bsig_d[:])
            ones32 = cpool.tile([1, P], F32)
            onesb = cpool.tile([1, P], BF)
            nc.sync.dma_start(ones32[:], ones32_d[:])
            nc.sync.dma_start(onesb[:], onesb_d[:])
            ident = cpool.tile([P, P], BF)
            make_identity(nc, ident[:])
            # distance^2 band tiles, one per row chunk (loaded in stage 1,
            # after the critical-path inputs)
            d2b = apool.tile([P, QC, 2 * P], F32)

            # ---- persistent activations ----
            qhT_s = apool.tile([P, IC, W], BF)   # [head_dim(o) chunks, queries]
            khT_s = apool.tile([P, IC, W], BF)
            vh_s = apool.tile([P, QC, D], BF)    # [key chunks, head_dim]
            attT_s = apool.tile([P, IC, W], BF)  # normalized att^T [e, i]
            a_s = apool.tile([P, QC, H], F32)    # -0.5/sigma^2  per (i, h)
            lnc_s = apool.tile([P, QC, H], F32)  # ln(1/(sqrt(2pi) sigma))
            rz_s = apool.tile([P, QC, H], F32)   # 1/Z per (i, h)
            # 1/Z transposed to rows, one tile per head (base partition 0)
            zrowT_hs = [apool.tile([1, W], BF, name=f"zrowT_{h}")
                        for h in range(H)]

            # ---- stage 1: projections + sigma ----
            with (
                tc.tile_pool(name="stg1_in", bufs=1) as inpool,
                tc.tile_pool(name="stg1_ps", bufs=2, space="PSUM") as pp1,
                tc.tile_pool(name="stg1_tmp", bufs=2) as tpool,
            ):
                qT_s = inpool.tile([P, IC, W], F32)
                qTb_s = inpool.tile([P, IC, W], BF)
                kTb_s = inpool.tile([P, IC, W], BF)
                vTb_s = inpool.tile([P, IC, W], BF)
                for kc in range(IC):
                    for t, d in ((qTb_s, qTb_d), (kTb_s, kTb_d),
                                 (vTb_s, vTb_d), (qT_s, qT_d)):
                        nc.sync.dma_start(
                            t[:, kc, :],
                            d.rearrange("(c p) f -> p c f", p=P)[:, kc, :])

                for ic in range(QC):
                    jlo, wd = band(ic)
                    nc.sync.dma_start(d2b[:, ic, :wd],
                                      dist2_d[ic * P:(ic + 1) * P, jlo:jlo + wd])

                # qhT / khT: [o, i] = Wx^T . xT ; bias added per-partition (o)
                for src, Wt, bvec, dst in ((qTb_s, Wq_s, bq_s, qhT_s),
                                           (kTb_s, Wk_s, bk_s, khT_s)):
                    for oc in range(IC):
                        for ih in range(2):
                            ps = pp1.tile([P, D], mybir.dt.float32, tag="ps")
                            for kc in range(IC):
                                mm(ps[:], Wt[:, kc, oc * P:(oc + 1) * P],
                                   src[:, kc, ih * D:(ih + 1) * D],
                                   start=(kc == 0), stop=(kc == IC - 1))
                            # copy-with-bias on DVE (ACT is the busy engine)
                            nc.vector.tensor_scalar(
                                dst[:, oc, ih * D:(ih + 1) * D], ps[:],
                                bvec[:, oc:oc + 1], None, mybir.AluOpType.add)

                # vh: [j, e] = vT^T . Wv + bv (bias via K=1 ones matmul)
                # bv is folded into bo on the host (sum_j series = 1)
                for jc in range(QC):
                    ps = pp1.tile([P, D], mybir.dt.float32, tag="ps")
                    for kc in range(IC):
                        mm(ps[:], vTb_s[:, kc, jc * P:(jc + 1) * P], Wv_s[:, kc, :],
                           start=(kc == 0), stop=(kc == IC - 1))
                    nc.vector.tensor_copy(vh_s[:, jc, :], ps[:])

                # sigma head (fp32): a = -0.5/sig^2, lnc = -ln(sig)-ln(sqrt(2pi))
                # one [128, QC*H] strip; activation funcs batched so the ACT
                # table is switched at most a few times
                sgs = tpool.tile([P, QC, H], F32, tag="sgs")
                for ic in range(QC):
                    ps = pp1.tile([P, H], mybir.dt.float32, tag="ps_sig")
                    for kc in range(IC):
                        mm(ps[:], qT_s[:, kc, ic * P:(ic + 1) * P], Wsig_s[:, kc, :],
                           start=(kc == 0), stop=False)
                    mm(ps[:], ones32[:, :P], bsig_row[:], start=False, stop=True)
                    nc.scalar.activation(sgs[:, ic, :], ps[:],
                                         ACTF.Sigmoid, scale=5.0)
                nc.vector.tensor_scalar_add(sgs[:], sgs[:], 1e-5)
                nc.scalar.activation(sgs[:], sgs[:], ACTF.Exp, scale=LN3)
                nc.vector.tensor_scalar_add(sgs[:], sgs[:], -1.0)  # sigma
                sq = tpool.tile([P, QC, H], F32, tag="sq")
                nc.vector.tensor_mul(sq[:], sgs[:], sgs[:])
                nc.vector.reciprocal(sq[:], sq[:])
                nc.vector.tensor_scalar_mul(a_s[:], sq[:], -0.5)
                nc.scalar.activation(sq[:], sgs[:], ACTF.Ln)
                nc.vector.tensor_scalar(lnc_s[:], sq[:], -1.0,
                                        -LN_SQRT_2PI,
                                        mybir.AluOpType.mult,
                                        mybir.AluOpType.add)

            # ---- phase A: scores (row) -> series + 1/Z; prior band ----
            with (
                tc.tile_pool(name="pa_ps", bufs=3, space="PSUM") as ppa,
                tc.tile_pool(name="pa_tp", bufs=2, space="PSUM") as ppt,
                tc.tile_pool(name="pa_sb", bufs=4) as spool,
            ):
                for ic in range(QC):
                    for h in range(H):
                        hp = (h % 2) * HS
                        ps = ppa.tile([P, W], mybir.dt.float32, tag="sc")
                        for jh in range(2):
                            mm(ps[:, jh * D:(jh + 1) * D],
                               qhT_s[hp:hp + HS, h // 2, ic * P:(ic + 1) * P],
                               khT_s[hp:hp + HS, h // 2, jh * D:(jh + 1) * D],
                               start=True, stop=True)
                        er = spool.tile([P, W], BF, tag="er")
                        zt = spool.tile([P, 1], F32, tag="zt")
                        nc.scalar.activation(er[:], ps[:], ACTF.Exp, scale=0.125,
                                             accum_out=zt[:])
                        nc.vector.reciprocal(rz_s[:, ic, h:h + 1], zt[:])
                        sr = spool.tile([P, W], BFO, tag="sr")
                        nc.vector.tensor_scalar_mul(sr[:], er[:],
                                                    rz_s[:, ic, h:h + 1])
                        nc.sync.dma_start(series_d[h, ic * P:(ic + 1) * P, :],
                                          sr[:])
                        # prior band for (ic, h); rest of the row is zero
                        jlo, wd = band(ic)
                        pr = spool.tile([P, 2 * P], BFO, tag="pr")
                        nc.scalar.activation(pr[:, :wd], d2b[:, ic, :wd],
                                             ACTF.Exp,
                                             scale=a_s[:, ic, h:h + 1],
                                             bias=lnc_s[:, ic, h:h + 1])
                        nc.sync.dma_start(
                            prior_d[h, ic * P:(ic + 1) * P, jlo:jlo + wd],
                            pr[:, :wd])
                    # transpose 1/Z rows: [128, 1] -> [1, 128] per head
                    # (fp16 operands so the PE transpose runs at 1 cyc/row)
                    rz16 = spool.tile([P, H], BF, tag="rz16")
                    nc.vector.tensor_copy(rz16[:], rz_s[:, ic, :])
                    for h in range(H):
                        tp = ppt.tile([1, P], BF, tag="tp")
                        nc.tensor.transpose(tp[:], rz16[:, h:h + 1], ident[:])
                        nc.vector.tensor_copy(
                            zrowT_hs[h][0:1, ic * P:(ic + 1) * P], tp[:])

            # ---- phase B: scores^T -> E^T -> att^T (normalized) ----
            with (
                tc.tile_pool(name="pb_ps", bufs=2, space="PSUM") as ppb,
                tc.tile_pool(name="pb_att", bufs=2, space="PSUM") as ppatt,
                tc.tile_pool(name="pb_sb", bufs=3) as bpool,
            ):
                for h in range(H):
                    hp = (h % 2) * HS
                    att_ps = ppatt.tile([HS, W], mybir.dt.float32, tag="att")
                    for jc in range(QC):
                        ps = ppb.tile([P, W], mybir.dt.float32, tag="scT")
                        for ih in range(2):
                            mm(ps[:, ih * D:(ih + 1) * D],
                               khT_s[hp:hp + HS, h // 2, jc * P:(jc + 1) * P],
                               qhT_s[hp:hp + HS, h // 2, ih * D:(ih + 1) * D],
                               start=True, stop=True)
                        et = bpool.tile([P, W], BF, tag="et")
                        nc.scalar.activation(et[:], ps[:], ACTF.Exp, scale=0.125)
                        for ih in range(2):
                            mm(att_ps[:, ih * D:(ih + 1) * D],
                               vh_s[:, jc, h * HS:(h + 1) * HS],
                               et[:, ih * D:(ih + 1) * D],
                               start=(jc == 0), stop=(jc == QC - 1))
                    # normalize: att^T[e, i] *= (1/Z)[i]  (broadcast over e)
                    zb_ps = ppb.tile([HS, W], mybir.dt.float32, tag="scT")
                    for ih in range(2):
                        mm(zb_ps[:, ih * D:(ih + 1) * D], onesb[:, :HS],
                           zrowT_hs[h][:, ih * D:(ih + 1) * D],
                           start=True, stop=True)
                    zb = bpool.tile([HS, W], F32, tag="zb")
                    nc.vector.tensor_copy(zb[:], zb_ps[:])
                    nc.vector.tensor_tensor(
                        attT_s[hp:hp + HS, h // 2, :], att_ps[:], zb[:],
                        mybir.AluOpType.mult)

            # ---- phase C: out = att . Wo + bo ----
            with (
                tc.tile_pool(name="pc_ps", bufs=2, space="PSUM") as ppc,
                tc.tile_pool(name="pc_sb", bufs=2) as opool,
            ):
                for ic in range(QC):
                    ps = ppc.tile([P, D], mybir.dt.float32, tag="out")
                    for ec in range(IC):
                        mm(ps[:], attT_s[:, ec, ic * P:(ic + 1) * P], Wo_s[:, ec, :],
                           start=(ec == 0), stop=False)
                    mm(ps[:], onesb[:, :P], bo_row[:], start=False, stop=True)
                    ot = opool.tile([P, D], F32, tag="ot")
                    nc.vector.tensor_copy(ot[:], ps[:])
                    nc.sync.dma_start(out_d[ic * P:(ic + 1) * P, :], ot[:])

    _split_multi_waits(nc)
    return nc


def make_in_maps(q, k, v, Wq, bq, Wk, bk, Wv, bv, Wsig, bsig, Wo, bo):
    BF = np.float16
    idx = np.arange(W, dtype=np.float32)
    dist2 = (idx[:, None] - idx[None, :]) ** 2

    def f32(x):
        return np.ascontiguousarray(x, dtype=np.float32)

    def bf(x):
        return np.ascontiguousarray(np.asarray(x, dtype=np.float32).astype(BF))

    shared = {
        "Wq": bf(Wq), "Wk": bf(Wk), "Wv": bf(Wv), "Wo": bf(Wo),
        "Wsig": f32(Wsig),
        "bq": f32(bq), "bk": f32(bk),
        # bv folded: out = att0@Wo + (bv@Wo + bo)  since sum_j series = 1
        "bo": bf(np.asarray(bv, np.float64) @ np.asarray(Wo, np.float64)
                 + np.asarray(bo, np.float64))[None, :],
        "bsig": f32(bsig)[None, :],
        "dist2": dist2,
        "ones32": np.ones((1, P), np.float32),
        "onesb": np.ones((1, P), BF),
    }
    in_maps = []
    for i in range(B):
        m = dict(shared)
        m["qT"] = f32(q[i].T)
        m["qTb"] = bf(m["qT"])
        m["kTb"] = bf(np.asarray(k[i].T, np.float32))
        m["vTb"] = bf(np.asarray(v[i].T, np.float32))
        in_maps.append(m)
    return in_maps


def run(nc, in_maps, **kw):
    from concourse.bass_utils import run_bass_kernel_spmd

    res = run_bass_kernel_spmd(nc, in_maps, list(range(B)), **kw)
    out = np.stack([np.asarray(res.results[i]["out"]) for i in range(B)])
    series = np.stack(
        [np.asarray(res.results[i]["series"]).astype(np.float32)
         for i in range(B)])
    prior = np.stack(
        [np.asarray(res.results[i]["prior"]).astype(np.float32)
         for i in range(B)])
    return (out, series, prior), res


def kernel(q, k, v, Wq, bq, Wk, bk, Wv, bv, Wsig, bsig, Wo, bo):
    _ensure_path()
    in_maps = make_in_maps(q, k, v, Wq, bq, Wk, bk, Wv, bv, Wsig, bsig, Wo, bo)
    nc = build_nc()
    outs, _ = run(nc, in_maps)
    return outs


# revision 37
# speedup vs baseline: 1.1863x; 1.1863x over previous
"""AnomalyAttention on 8 Trainium2 NeuronCores.

Batch-parallel: core i computes batch element i (B=8). No collectives.
Host-side prep: transpose q/k/v per batch element (plus bf16 copies for
the TensorEngine) and precompute the (i-j)^2 distance table; everything
else (projections, sigma head, scores, softmax, Gaussian prior,
attention, output projection) runs on device.

Matmuls run in bf16 with fp32 PSUM accumulation.  The sigma head
(which the prior is very sensitive to) runs fully in fp32.  series and
prior are produced as bf16 in DRAM and upcast to fp32 on the host.

Outputs (matching the reference tuple): out [8,1024,512],
series [8,8,1024,1024], prior [8,8,1024,1024], all float32.
"""

import math
import sys

import numpy as np

B, W, D, H = 8, 1024, 512, 8
HS = D // H          # 64
P = 128
IC = D // P          # 4 contraction chunks over D
QC = W // P          # 8 row chunks over W
LN3 = math.log(3.0)
LN_SQRT_2PI = 0.9189385332046727  # ln(sqrt(2*pi))


def _ensure_path():
    for p in ("/opt/trn_rl_repo", "/root/.axon_site/_ro/trn_rl_repo"):
        if p not in sys.path:
            sys.path.append(p)


def _patch_tile_drain():
    """Walrus in this container rejects the multi-wait Drain that
    TileContext emits at exit ("Too many sync wait commands").  Split the
    final waits across single-wait NOPs on the sync engine instead."""
    import concourse.tile as tile
    from concourse.vector_clock import ScopedClock, VectorClock

    if getattr(tile.TileContext, "_drain_patched", False):
        return

    def _drain_and_barrier(self, tick_clock, wait_clock):
        vc = tick_clock.global_clock
        for i in range(len(vc)):
            if vc[i] <= 0:
                continue
            sub = VectorClock([vc[j] if j == i else 0 for j in range(len(vc))])
            nop = self.nc.sync.nop(nofuse=True, hint="drain_split")
            wait_clock.add_sem_waits(nop.ins, ScopedClock({None: sub}))
        self.nc.sync.drain()
        self.nc.all_engine_barrier()
        assert self.sems is not None
        popped = self.nc._tile_sem_poison_stack.pop()
        assert popped is self._sem_poison
        self.nc.clear_and_free_semaphores(list(self.sems.allocated().values()))
        self.nc.all_engine_barrier()

    tile.TileContext._drain_and_barrier = _drain_and_barrier
    tile.TileContext._drain_patched = True


def _split_multi_waits(nc):
    """This container's walrus allows at most one sync-wait per
    instruction.  Hoist extra waits onto NOPs inserted just before, on the
    same engine."""
    import concourse.mybir as mybir

    n_split = 0
    for f in nc.m.functions:
        for bb in f.blocks:
            insts = bb.instructions
            i = 0
            while i < len(insts):
                inst = insts[i]
                si = inst.sync_info
                if si is not None and si.on_wait and len(si.on_wait) > 1:
                    waits = list(si.on_wait)
                    nops = []
                    for j, wt in enumerate(waits[:-1]):
                        nop = mybir.InstNoOp(name=f"{inst.name}-wsplit{j}")
                        nop.engine = inst.engine
                        nop.sync_info = mybir.SyncInfo(on_wait=[wt],
                                                       on_update=[])
                        nops.append(nop)
                    inst.sync_info = mybir.SyncInfo(
                        on_wait=[waits[-1]], on_update=list(si.on_update or []))
                    for j, nop in enumerate(nops):
                        insts.insert(i + j, nop)
                    i += len(nops)
                    n_split += 1
                i += 1
    return n_split


def _patch_ldw_opt():
    """Re-enable walrus's LDWEIGHTS dedup (disabled by default in this
    container's compile driver)."""
    import concourse.bass_utils as bu

    if getattr(bu, "_ldw_patched", False):
        return
    orig = bu.run_command

    def run_command(cmd, *a, **kw):
        return orig(cmd, *a, **kw)

    bu.run_command = run_command
    bu._ldw_patched = True


def _dedup_ldweights(nc):
    """Remove back-to-back duplicate LDWEIGHTS on the PE stream (walrus's
    ldw-opt pass is disabled in this container and crashes when enabled).
    Safe only when the duplicate carries no sync conditions."""
    import concourse.mybir as mybir

    n_rm = 0
    for f in nc.m.functions:
        for bb in f.blocks:
            insts = bb.instructions
            last_ldw_key = None
            keep = []
            for inst in insts:
                if inst.engine == mybir.EngineType.PE:
                    op = type(inst).__name__
                    if op == "InstLdweights":
                        try:
                            pa = inst.ins[0]
                            key = (pa.memref, pa.offset, str(pa.ap),
                                   str(pa.dtype), str(inst.perf_mode),
                                   str(inst.is_transpose))
                        except Exception:
                            key = None
                        si = inst.sync_info
                        clean = not (si and (si.on_wait or si.on_update))
                        if clean and key is not None and key == last_ldw_key:
                            n_rm += 1
                            continue  # drop duplicate
                        last_ldw_key = key
                    elif op != "InstMatmult":
                        last_ldw_key = None
                keep.append(inst)
            if len(keep) != len(insts):
                insts[:] = keep
    return n_rm


def build_nc():
    """Build the single-core Bass graph (SPMD: identical on all 8 cores)."""
    _ensure_path()
    _patch_tile_drain()
    _patch_ldw_opt()

    import concourse.bass as bass
    import concourse.mybir as mybir
    import concourse.tile as tile
    from concourse.masks import make_identity

    F32 = mybir.dt.float32
    BF = mybir.dt.float16       # TensorEngine operand dtype (full rate, 10-bit mantissa)
    BFO = mybir.dt.bfloat16     # series/prior DRAM output dtype (wide exponent)
    ACTF = mybir.ActivationFunctionType

    nc = bass.Bass()

    # fp32 sigma path input
    qT_d = nc.declare_dram_parameter("qT", [D, W], F32, isOutput=False)
    # bf16 TensorEngine inputs
    qTb_d = nc.declare_dram_parameter("qTb", [D, W], BF, isOutput=False)
    kTb_d = nc.declare_dram_parameter("kTb", [D, W], BF, isOutput=False)
    vTb_d = nc.declare_dram_parameter("vTb", [D, W], BF, isOutput=False)
    Wq_d = nc.declare_dram_parameter("Wq", [D, D], BF, isOutput=False)
    Wk_d = nc.declare_dram_parameter("Wk", [D, D], BF, isOutput=False)
    Wv_d = nc.declare_dram_parameter("Wv", [D, D], BF, isOutput=False)
    Wo_d = nc.declare_dram_parameter("Wo", [D, D], BF, isOutput=False)
    Wsig_d = nc.declare_dram_parameter("Wsig", [D, H], F32, isOutput=False)
    bq_d = nc.declare_dram_parameter("bq", [D], F32, isOutput=False)
    bk_d = nc.declare_dram_parameter("bk", [D], F32, isOutput=False)
    bo_d = nc.declare_dram_parameter("bo", [1, D], BF, isOutput=False)
    bsig_d = nc.declare_dram_parameter("bsig", [1, H], F32, isOutput=False)
    dist2_d = nc.declare_dram_parameter("dist2", [W, W], F32, isOutput=False)
    ones32_d = nc.declare_dram_parameter("ones32", [1, P], F32, isOutput=False)
    onesb_d = nc.declare_dram_parameter("onesb", [1, P], BF, isOutput=False)

    out_d = nc.declare_dram_parameter("out", [W, D], F32, isOutput=True)
    series_d = nc.declare_dram_parameter("series", [H, W, W], BFO, isOutput=True)
    prior_d = nc.declare_dram_parameter("prior", [H, W, W], BFO, isOutput=True)

    mm = nc.tensor.matmul

    # prior is exactly zero (f32 underflow) for |i-j| > ~29 (sigma <= 2.00003);
    # compute only a +-64 band per 128-row chunk, the rest of each row stays
    # at the pre-zeroed output buffer value.
    def band(ic):
        jlo = max(0, ic * P - HS)
        jhi = min(W, (ic + 1) * P + HS)
        return jlo, jhi - jlo

    with tile.TileContext(nc) as tc:
        with (
            tc.tile_pool(name="const", bufs=1) as cpool,
            tc.tile_pool(name="acts", bufs=1) as apool,
        ):
            # ---- persistent constants ----
            Wq_s = cpool.tile([P, IC, D], BF)
            Wk_s = cpool.tile([P, IC, D], BF)
            Wv_s = cpool.tile([P, IC, D], BF)
            Wo_s = cpool.tile([P, IC, D], BF)
            Wsig_s = cpool.tile([P, IC, H], F32)
            for t, d in ((Wq_s, Wq_d), (Wk_s, Wk_d), (Wv_s, Wv_d), (Wo_s, Wo_d),
                         (Wsig_s, Wsig_d)):
                nc.sync.dma_start(t[:], d.rearrange("(c p) f -> p c f", p=P))
            bq_s = cpool.tile([P, IC], F32)
            bk_s = cpool.tile([P, IC], F32)
            nc.sync.dma_start(bq_s[:], bq_d.rearrange("(c p) -> p c", p=P))
            nc.sync.dma_start(bk_s[:], bk_d.rearrange("(c p) -> p c", p=P))
            bo_row = cpool.tile([1, D], BF)
            bsig_row = cpool.tile([1, H], F32)
            nc.sync.dma_start(bo_row[:], bo_d[:])
            nc.sync.dma_start(bsig_row[:], bsig_d[:])
            ones32 = cpool.tile([1, P], F32)
            onesb = cpool.tile([1, P], BF)
            nc.sync.dma_start(ones32[:], ones32_d[:])
            nc.sync.dma_start(onesb[:], onesb_d[:])
            ident = cpool.tile([P, P], F32)
            make_identity(nc, ident[:])
            # distance^2 band tiles, one per row chunk (loaded in stage 1,
            # after the critical-path inputs)
            d2b = apool.tile([P, QC, 2 * P], F32)

            # ---- persistent activations ----
            qhT_s = apool.tile([P, IC, W], BF)   # [head_dim(o) chunks, queries]
            khT_s = apool.tile([P, IC, W], BF)
            vh_s = apool.tile([P, QC, D], BF)    # [key chunks, head_dim]
            attT_s = apool.tile([P, IC, W], BF)  # normalized att^T [e, i]
            a_s = apool.tile([P, QC, H], F32)    # -0.5/sigma^2  per (i, h)
            lnc_s = apool.tile([P, QC, H], F32)  # ln(1/(sqrt(2pi) sigma))
            rz_s = apool.tile([P, QC, H], F32)   # 1/Z per (i, h)
            # 1/Z transposed to rows, one tile per head (base partition 0)
            zrowT_hs = [apool.tile([1, W], BF, name=f"zrowT_{h}")
                        for h in range(H)]

            # ---- stage 1: projections + sigma ----
            with (
                tc.tile_pool(name="stg1_in", bufs=1) as inpool,
                tc.tile_pool(name="stg1_ps", bufs=2, space="PSUM") as pp1,
                tc.tile_pool(name="stg1_tmp", bufs=2) as tpool,
            ):
                qT_s = inpool.tile([P, IC, W], F32)
                qTb_s = inpool.tile([P, IC, W], BF)
                kTb_s = inpool.tile([P, IC, W], BF)
                vTb_s = inpool.tile([P, IC, W], BF)
                for kc in range(IC):
                    for t, d in ((qTb_s, qTb_d), (kTb_s, kTb_d),
                                 (vTb_s, vTb_d), (qT_s, qT_d)):
                        nc.sync.dma_start(
                            t[:, kc, :],
                            d.rearrange("(c p) f -> p c f", p=P)[:, kc, :])

                for ic in range(QC):
                    jlo, wd = band(ic)
                    nc.sync.dma_start(d2b[:, ic, :wd],
                                      dist2_d[ic * P:(ic + 1) * P, jlo:jlo + wd])

                # qhT / khT: [o, i] = Wx^T . xT ; bias added per-partition (o)
                for src, Wt, bvec, dst in ((qTb_s, Wq_s, bq_s, qhT_s),
                                           (kTb_s, Wk_s, bk_s, khT_s)):
                    for oc in range(IC):
                        for ih in range(2):
                            ps = pp1.tile([P, D], mybir.dt.float32, tag="ps")
                            for kc in range(IC):
                                mm(ps[:], Wt[:, kc, oc * P:(oc + 1) * P],
                                   src[:, kc, ih * D:(ih + 1) * D],
                                   start=(kc == 0), stop=(kc == IC - 1))
                            # copy-with-bias on DVE (ACT is the busy engine)
                            nc.vector.tensor_scalar(
                                dst[:, oc, ih * D:(ih + 1) * D], ps[:],
                                bvec[:, oc:oc + 1], None, mybir.AluOpType.add)

                # vh: [j, e] = vT^T . Wv + bv (bias via K=1 ones matmul)
                # bv is folded into bo on the host (sum_j series = 1)
                for jc in range(QC):
                    ps = pp1.tile([P, D], mybir.dt.float32, tag="ps")
                    for kc in range(IC):
                        mm(ps[:], vTb_s[:, kc, jc * P:(jc + 1) * P], Wv_s[:, kc, :],
                           start=(kc == 0), stop=(kc == IC - 1))
                    nc.vector.tensor_copy(vh_s[:, jc, :], ps[:])

                # sigma head (fp32): a = -0.5/sig^2, lnc = -ln(sig)-ln(sqrt(2pi))
                # one [128, QC*H] strip; activation funcs batched so the ACT
                # table is switched at most a few times
                sgs = tpool.tile([P, QC, H], F32, tag="sgs")
                for ic in range(QC):
                    ps = pp1.tile([P, H], mybir.dt.float32, tag="ps_sig")
                    for kc in range(IC):
                        mm(ps[:], qT_s[:, kc, ic * P:(ic + 1) * P], Wsig_s[:, kc, :],
                           start=(kc == 0), stop=False)
                    mm(ps[:], ones32[:, :P], bsig_row[:], start=False, stop=True)
                    nc.scalar.activation(sgs[:, ic, :], ps[:],
                                         ACTF.Sigmoid, scale=5.0)
                nc.vector.tensor_scalar_add(sgs[:], sgs[:], 1e-5)
                nc.scalar.activation(sgs[:], sgs[:], ACTF.Exp, scale=LN3)
                nc.vector.tensor_scalar_add(sgs[:], sgs[:], -1.0)  # sigma
                sq = tpool.tile([P, QC, H], F32, tag="sq")
                nc.vector.tensor_mul(sq[:], sgs[:], sgs[:])
                nc.vector.reciprocal(sq[:], sq[:])
                nc.vector.tensor_scalar_mul(a_s[:], sq[:], -0.5)
                nc.scalar.activation(sq[:], sgs[:], ACTF.Ln)
                nc.vector.tensor_scalar(lnc_s[:], sq[:], -1.0,
                                        -LN_SQRT_2PI,
                                        mybir.AluOpType.mult,
                                        mybir.AluOpType.add)

            # ---- phase A: scores (row) -> series + 1/Z; prior band ----
            with (
                tc.tile_pool(name="pa_ps", bufs=3, space="PSUM") as ppa,
                tc.tile_pool(name="pa_tp", bufs=2, space="PSUM") as ppt,
                tc.tile_pool(name="pa_sb", bufs=4) as spool,
            ):
                for ic in range(QC):
                    for h in range(H):
                        hp = (h % 2) * HS
                        ps = ppa.tile([P, W], mybir.dt.float32, tag="sc")
                        for jh in range(2):
                            mm(ps[:, jh * D:(jh + 1) * D],
                               qhT_s[hp:hp + HS, h // 2, ic * P:(ic + 1) * P],
                               khT_s[hp:hp + HS, h // 2, jh * D:(jh + 1) * D],
                               start=True, stop=True)
                        er = spool.tile([P, W], BF, tag="er")
                        zt = spool.tile([P, 1], F32, tag="zt")
                        nc.scalar.activation(er[:], ps[:], ACTF.Exp, scale=0.125,
                                             accum_out=zt[:])
                        nc.vector.reciprocal(rz_s[:, ic, h:h + 1], zt[:])
                        sr = spool.tile([P, W], BFO, tag="sr")
                        nc.vector.tensor_scalar_mul(sr[:], er[:],
                                                    rz_s[:, ic, h:h + 1])
                        nc.sync.dma_start(series_d[h, ic * P:(ic + 1) * P, :],
                                          sr[:])
                        # prior band for (ic, h); rest of the row is zero
                        jlo, wd = band(ic)
                        pr = spool.tile([P, 2 * P], BFO, tag="pr")
                        nc.scalar.activation(pr[:, :wd], d2b[:, ic, :wd],
                                             ACTF.Exp,
                                             scale=a_s[:, ic, h:h + 1],
                                             bias=lnc_s[:, ic, h:h + 1])
                        nc.sync.dma_start(
                            prior_d[h, ic * P:(ic + 1) * P, jlo:jlo + wd],
                            pr[:, :wd])
                    # transpose 1/Z rows: [128, 1] -> [1, 128] per head
                    for h in range(H):
                        tp = ppt.tile([1, P], mybir.dt.float32, tag="tp")
                        nc.tensor.transpose(tp[:], rz_s[:, ic, h:h + 1], ident[:])
                        nc.vector.tensor_copy(
                            zrowT_hs[h][0:1, ic * P:(ic + 1) * P], tp[:])

            # ---- phase B: scores^T -> E^T -> att^T (normalized) ----
            with (
                tc.tile_pool(name="pb_ps", bufs=2, space="PSUM") as ppb,
                tc.tile_pool(name="pb_att", bufs=2, space="PSUM") as ppatt,
                tc.tile_pool(name="pb_sb", bufs=3) as bpool,
            ):
                for h in range(H):
                    hp = (h % 2) * HS
                    att_ps = ppatt.tile([HS, W], mybir.dt.float32, tag="att")
                    for jc in range(QC):
                        ps = ppb.tile([P, W], mybir.dt.float32, tag="scT")
                        for ih in range(2):
                            mm(ps[:, ih * D:(ih + 1) * D],
                               khT_s[hp:hp + HS, h // 2, jc * P:(jc + 1) * P],
                               qhT_s[hp:hp + HS, h // 2, ih * D:(ih + 1) * D],
                               start=True, stop=True)
                        et = bpool.tile([P, W], BF, tag="et")
                        nc.scalar.activation(et[:], ps[:], ACTF.Exp, scale=0.125)
                        for ih in range(2):
                            mm(att_ps[:, ih * D:(ih + 1) * D],
                               vh_s[:, jc, h * HS:(h + 1) * HS],
                               et[:, ih * D:(ih + 1) * D],
                               start=(jc == 0), stop=(jc == QC - 1))
                    # normalize: att^T[e, i] *= (1/Z)[i]  (broadcast over e)
                    zb_ps = ppb.tile([HS, W], mybir.dt.float32, tag="scT")
                    for ih in range(2):
                        mm(zb_ps[:, ih * D:(ih + 1) * D], onesb[:, :HS],
                           zrowT_hs[h][:, ih * D:(ih + 1) * D],
                           start=True, stop=True)
                    zb = bpool.tile([HS, W], F32, tag="zb")
                    nc.vector.tensor_copy(zb[:], zb_ps[:])
                    nc.vector.tensor_tensor(
                        attT_s[hp:hp + HS, h // 2, :], att_ps[:], zb[:],
                        mybir.AluOpType.mult)

            # ---- phase C: out = att . Wo + bo ----
            with (
                tc.tile_pool(name="pc_ps", bufs=2, space="PSUM") as ppc,
                tc.tile_pool(name="pc_sb", bufs=2) as opool,
            ):
                for ic in range(QC):
                    ps = ppc.tile([P, D], mybir.dt.float32, tag="out")
                    for ec in range(IC):
                        mm(ps[:], attT_s[:, ec, ic * P:(ic + 1) * P], Wo_s[:, ec, :],
                           start=(ec == 0), stop=False)
                    mm(ps[:], onesb[:, :P], bo_row[:], start=False, stop=True)
                    ot = opool.tile([P, D], F32, tag="ot")
                    nc.vector.tensor_copy(ot[:], ps[:])
                    nc.sync.dma_start(out_d[ic * P:(ic + 1) * P, :], ot[:])

    _split_multi_waits(nc)
    return nc


def make_in_maps(q, k, v, Wq, bq, Wk, bk, Wv, bv, Wsig, bsig, Wo, bo):
    BF = np.float16
    idx = np.arange(W, dtype=np.float32)
    dist2 = (idx[:, None] - idx[None, :]) ** 2

    def f32(x):
        return np.ascontiguousarray(x, dtype=np.float32)

    def bf(x):
        return np.ascontiguousarray(np.asarray(x, dtype=np.float32).astype(BF))

    shared = {
        "Wq": bf(Wq), "Wk": bf(Wk), "Wv": bf(Wv), "Wo": bf(Wo),
        "Wsig": f32(Wsig),
        "bq": f32(bq), "bk": f32(bk),
        # bv folded: out = att0@Wo + (bv@Wo + bo)  since sum_j series = 1
        "bo": bf(np.asarray(bv, np.float64) @ np.asarray(Wo, np.float64)
                 + np.asarray(bo, np.float64))[None, :],
        "bsig": f32(bsig)[None, :],
        "dist2": dist2,
        "ones32": np.ones((1, P), np.float32),
        "onesb": np.ones((1, P), BF),
    }
    in_maps = []
    for i in range(B):
        m = dict(shared)
        m["qT"] = f32(q[i].T)
        m["qTb"] = bf(m["qT"])
        m["kTb"] = bf(np.asarray(k[i].T, np.float32))
        m["vTb"] = bf(np.asarray(v[i].T, np.float32))
        in_maps.append(m)
    return in_maps


def run(nc, in_maps, **kw):
    from concourse.bass_utils import run_bass_kernel_spmd

    res = run_bass_kernel_spmd(nc, in_maps, list(range(B)), **kw)
    out = np.stack([np.asarray(res.results[i]["out"]) for i in range(B)])
    series = np.stack(
        [np.asarray(res.results[i]["series"]).astype(np.float32)
         for i in range(B)])
    prior = np.stack(
        [np.asarray(res.results[i]["prior"]).astype(np.float32)
         for i in range(B)])
    return (out, series, prior), res


def kernel(q, k, v, Wq, bq, Wk, bk, Wv, bv, Wsig, bsig, Wo, bo):
    _ensure_path()
    in_maps = make_in_maps(q, k, v, Wq, bq, Wk, bk, Wv, bv, Wsig, bsig, Wo, bo)
    nc = build_nc()
    outs, _ = run(nc, in_maps)
    return outs
